# revision 26
# baseline (speedup 1.0000x reference)
"""Bass/Trainium2 kernel for 2-layer GAT (nn_GATa_45260365365735).

Strategy (8 NeuronCores, SPMD, three launches; host does only
indexing/duplication between launches — all arithmetic on device):

  - Nodes assigned to cores round-robin by global in-degree rank (balanced
    edges + identical block shapes).  Each core owns all edges targeting its
    nodes; segment softmax + aggregation are core-local.  Owned nodes form
    128-lane blocks; lane (p, b) holds its node's in-edges in chunk columns.
  - Algebraic collapse: layer 2 only consumes h2 = h1 @ W2; by linearity a
    node only needs p = x @ wsc with 12 outputs: e_src(4) | z(4) | e_dst(4),
    where z folds W2 into W1.
  - Launch 0: p per NODE ([N,128] @ [128,12] bf16) — removes the per-edge
    duplication of 128-dim features (23 MB/core -> 3.2 MB/core HBM).
  - Host gathers per-slot fp16 streams in PLANE-MAJOR, GROUP-PACKED layout:
    every elementwise op and segmented reduction runs on contiguous data.
  - Launch 1 (edge pass) engine split tuned to measured rates: gpsimd does
    u = e_src + e_dst; the scalar engine does ALL Prelus then ALL Exps (two
    activation-table loads total; fp16 outputs); vector does w*z (fp16, 2x
    rate) + per-run 4D segmented reductions (runs of equal chunk count C are
    contiguous since CB is non-increasing) + the epilogue with
    reciprocal_approx_fast.  Pad slots: host writes e_src = -60000 -> w = 0.
  - Launch 2 repeats the pattern on h2 streams (1 "head"); output written
    contiguously as [P, NB] (host inverse-permutes).
"""

import os
import numpy as np
import ml_dtypes

P = 128
N_CORES = 8
HEADS = 4
HID = 32
IN_DIM = 128
NEG_SLOPE = 0.2
EPS = 1e-16
PW = 12            # per-node payload: e_src(4) | z(4) | e_dst(4)
NEG_BIG = -60000.0   # pad-slot e_src kill value (fp16-safe)

_COMPILED = {}
LAST_EXEC_NS = None
LAST_RESULTS = None


# --------------------------------------------------------------------------
# host preprocessing (indexing only)
# --------------------------------------------------------------------------

def _preprocess(x, edge_index, W1, att_src1, att_dst1, b1, W2, att_src2,
                att_dst2, b2, n_cores=None):
    if n_cores is None:
        n_cores = N_CORES
    N = x.shape[0]
    ei = np.asarray(edge_index).astype(np.int64)
    src = np.concatenate([ei[0], np.arange(N, dtype=np.int64)])
    dst = np.concatenate([ei[1], np.arange(N, dtype=np.int64)])

    deg = np.bincount(dst, minlength=N).astype(np.int64)
    order = np.argsort(dst, kind="stable")
    src_sorted = src[order]
    estart = np.concatenate([[0], np.cumsum(deg)]).astype(np.int64)

    grank = np.argsort(-deg, kind="stable")
    perms = [grank[c::n_cores] for c in range(n_cores)]
    LP = int(np.ceil(max(len(p) for p in perms) / P) * P)
    NB = LP // P
    for c in range(n_cores):
        pad = np.full(LP - len(perms[c]), -1, dtype=np.int64)
        perms[c] = np.concatenate([perms[c], pad])

    blockmax = np.zeros((n_cores, NB), dtype=np.int64)
    for c in range(n_cores):
        pids = perms[c]
        d = np.where(pids >= 0, deg[np.maximum(pids, 0)], 0)
        blockmax[c] = d.reshape(NB, P).max(axis=1)
    CB = np.maximum(blockmax.max(axis=0), 1).astype(np.int64)
    T1 = int(CB.sum())
    offs = np.concatenate([[0], np.cumsum(CB)]).astype(np.int64)

    xf = np.asarray(x, dtype=np.float32)
    cores = []
    for c in range(n_cores):
        pids = perms[c]
        safe_pids = np.maximum(pids, 0)
        sg = np.zeros((P, T1), dtype=np.int64)   # src node per slot
        dg = np.zeros((P, T1), dtype=np.int64)   # dst (lane) node per slot
        valid = np.zeros((P, T1), dtype=bool)
        for b in range(NB):
            C = int(CB[b])
            rows = pids[b * P:(b + 1) * P]
            safe = np.maximum(rows, 0)
            d = np.where(rows >= 0, deg[safe], 0)
            st = estart[safe]
            cols = np.arange(C, dtype=np.int64)[None, :]
            vb = cols < d[:, None]
            eix = np.clip(st[:, None] + cols, 0, src.shape[0] - 1)
            o0 = int(offs[b])
            sg[:, o0:o0 + C] = np.where(vb, src_sorted[eix], 0)
            dg[:, o0:o0 + C] = np.where(vb, safe[:, None], 0)
            valid[:, o0:o0 + C] = vb
        xTc = np.ascontiguousarray(
            xf[safe_pids].T).astype(ml_dtypes.bfloat16)   # [IN_DIM, LP]
        cores.append(dict(xT=xTc, sg=sg, dg=dg, valid=valid, pids=pids))

    W1 = np.asarray(W1, dtype=np.float32)
    a_s1 = np.asarray(att_src1, dtype=np.float32)
    a_d1 = np.asarray(att_dst1, dtype=np.float32)
    W2v = np.asarray(W2, dtype=np.float32).reshape(-1)
    W1a = np.einsum("khc,hc->kh", W1.reshape(IN_DIM, HEADS, HID), a_s1)
    W1b = np.einsum("khc,hc->kh", W1.reshape(IN_DIM, HEADS, HID), a_d1)
    W12h = np.einsum("khf,hf->kh", W1.reshape(IN_DIM, HEADS, HID),
                     W2v.reshape(HEADS, HID))
    wsc = np.concatenate([W1a, W12h, W1b], axis=1).astype(ml_dtypes.bfloat16)
    b1v = np.asarray(b1, dtype=np.float32).reshape(-1)
    c0 = float(b1v @ W2v)
    screp = np.zeros((P, 4), dtype=np.float32)
    screp[:, 0] = float(np.asarray(att_src2).reshape(-1)[0])
    screp[:, 1] = float(np.asarray(att_dst2).reshape(-1)[0])
    screp[:, 2] = float(np.asarray(b2).reshape(-1)[0])
    screp[:, 3] = c0

    meta = dict(N=N, LP=LP, NB=NB, T1=T1, CB=CB.tolist(),
                offs=offs.tolist(), n_cores=n_cores)
    shared = dict(wsc=wsc, screp=screp)
    return meta, shared, cores


def _runs(CB):
    """Contiguous runs of equal chunk count (CB is non-increasing)."""
    runs = []
    b = 0
    NB = len(CB)
    while b < NB:
        e = b
        while e < NB and CB[e] == CB[b]:
            e += 1
        runs.append((b, e, CB[b]))
        b = e
    return runs


def _groups(runs, T1, n, first_frac=1.0):
    """Split runs into ~n groups (run-aligned); first group scaled by
    first_frac so the compute pipeline starts on a smaller DMA."""
    weights = [first_frac] + [1.0] * (n - 1)
    wsum = sum(weights)
    groups, cur, tot = [], [], 0
    gi = 0
    tgt = T1 * weights[0] / wsum
    for r in runs:
        cur.append(r)
        tot += (r[1] - r[0]) * r[2]
        if tot >= tgt and gi < n - 1:
            groups.append(cur)
            cur, tot = [], 0
            gi += 1
            tgt = T1 * weights[gi] / wsum
    if cur:
        groups.append(cur)
    return groups


def _l0_groups(NB):
    sizes = []
    left = NB
    for s in [8, 18, 24]:
        if left <= 0:
            break
        sizes.append(min(s, left))
        left -= sizes[-1]
    while left > 0:
        s = min(24, left)
        sizes.append(s)
        left -= s
    return sizes


# --------------------------------------------------------------------------
# launch 0: per-node p = x @ wsc   (x sharded by owned nodes)
# --------------------------------------------------------------------------

def _build_l0(meta, psum_dma=True):
    from contextlib import ExitStack
    import concourse.tile as tile
    from concourse import bacc, mybir

    LP, NB = meta["LP"], meta["NB"]
    n_cores = meta["n_cores"]
    f32, bf16 = mybir.dt.float32, mybir.dt.bfloat16

    nc = bacc.Bacc("TRN2", target_bir_lowering=False, debug=False,
                   enable_asserts=False, num_devices=n_cores)
    t_xT = nc.dram_tensor("xT", [IN_DIM, LP], bf16, kind="ExternalInput")
    t_wsc = nc.dram_tensor("wsc", [IN_DIM, PW], bf16, kind="ExternalInput")
    t_p = nc.dram_tensor("p", [P, NB * PW], f32, kind="ExternalOutput")

    sizes = _l0_groups(NB)

    with tile.TileContext(nc) as tc, ExitStack() as ctx:
        consts = ctx.enter_context(tc.tile_pool(name="consts", bufs=1))
        wsct = consts.tile([IN_DIM, PW], bf16)
        nc.sync.dma_start(wsct[:], t_wsc.ap())

        xp = ctx.enter_context(tc.tile_pool(name="xp", bufs=1))
        pp = ctx.enter_context(tc.tile_pool(name="pp", bufs=4, space="PSUM"))
        op = ctx.enter_context(tc.tile_pool(name="op", bufs=1))

        # issue every input DMA first so the sync queue never stalls on
        # an output DMA that waits for compute
        xts = []
        b0 = 0
        for gi, nbg in enumerate(sizes):
            xt = xp.tile([P, nbg * P], bf16, tag=f"xt{gi}", name=f"xt{gi}")
            nc.sync.dma_start(
                xt[:], t_xT.ap()[:, b0 * P:(b0 + nbg) * P])
            xts.append((xt, b0, nbg))
            b0 += nbg

        for gi, (xt, b0, nbg) in enumerate(xts):
            ps = pp.tile([P, nbg * PW], f32, tag="ps",
                         padded_shape=[P, 24 * PW], name=f"ps{gi}")
            for j in range(nbg):
                nc.tensor.matmul(ps[:, j * PW:(j + 1) * PW],
                                 lhsT=xt[:, j * P:(j + 1) * P],
                                 rhs=wsct[:], start=True, stop=True)
            if psum_dma:
                nc.sync.dma_start(
                    t_p.ap()[:, b0 * PW:(b0 + nbg) * PW], ps[:, 0:nbg * PW])
            else:
                po = op.tile([P, nbg * PW], f32, tag=f"po{gi}",
                             name=f"po{gi}")
                nc.vector.tensor_copy(po[:], ps[:, 0:nbg * PW])
                nc.sync.dma_start(
                    t_p.ap()[:, b0 * PW:(b0 + nbg) * PW], po[:])

    nc.compile()
    return nc


# --------------------------------------------------------------------------
# launch 1: per-slot p streams -> per-node h2 (layer-1 softmax + aggregate)
# --------------------------------------------------------------------------

def _build_l1(meta):
    from contextlib import ExitStack
    import concourse.tile as tile
    from concourse import bacc, mybir

    NB, T1 = meta["NB"], meta["T1"]
    CB, offs = meta["CB"], meta["offs"]
    n_cores = meta["n_cores"]
    f32, fp16 = mybir.dt.float32, mybir.dt.float16
    H = HEADS

    nc = bacc.Bacc("TRN2", target_bir_lowering=False, debug=False,
                   enable_asserts=False, num_devices=n_cores)
    t_S = nc.dram_tensor("S", [P, T1 * PW], fp16, kind="ExternalInput")
    t_sc = nc.dram_tensor("screp", [P, 4], f32, kind="ExternalInput")
    t_h2 = nc.dram_tensor("h2", [P, NB], f32, kind="ExternalOutput")

    groups = _groups(_runs(CB), T1, 5, first_frac=0.5)

    with tile.TileContext(nc) as tc, ExitStack() as ctx:
        consts = ctx.enter_context(tc.tile_pool(name="consts", bufs=1))
        sc = consts.tile([P, 4], f32)
        nc.sync.dma_start(sc[:], t_sc.ap())
        alf = consts.tile([P, 1], f32)       # Prelu negative slope
        nc.vector.tensor_scalar(alf[:], sc[0:P, 0:1], 0.0, NEG_SLOPE,
                                op0=mybir.AluOpType.mult,
                                op1=mybir.AluOpType.add)
        s8 = consts.tile([P, 8 * NB], f32)   # block-major [num(4) | den(4)]
        s8v = s8[:].rearrange("p (b f) -> p b f", b=NB, f=8)

        sp = ctx.enter_context(tc.tile_pool(name="sp", bufs=1))

        tiles = []
        # phase A: DMA + u-add (gpsimd) + Prelu (scalar, one table)
        for gi, grp in enumerate(groups):
            b0g, b1g = grp[0][0], grp[-1][1]
            o0g, o1g = offs[b0g], offs[b1g]
            W = o1g - o0g
            Sg = sp.tile([P, W * PW], fp16, tag=f"S{gi}", name=f"S{gi}")
            nc.sync.dma_start(Sg[:], t_S.ap()[:, o0g * PW:o1g * PW])
            U = sp.tile([P, 4 * W], fp16, tag=f"U{gi}", name=f"U{gi}")
            nc.vector.tensor_tensor(U[:], Sg[:, 0:4 * W], Sg[:, 8 * W:12 * W],
                                    op=mybir.AluOpType.add)
            LR = sp.tile([P, 4 * W], fp16, tag=f"LR{gi}", name=f"LR{gi}")
            nc.scalar.activation(LR[:], U[:],
                                 mybir.ActivationFunctionType.Prelu,
                                 alpha=alf[:])
            tiles.append((grp, o0g, W, Sg, LR))

        # phase B: Exp (scalar, one table) + w*z (vector) + reductions
        for gi, (grp, o0g, W, Sg, LR) in enumerate(tiles):
            WZ = sp.tile([P, 8 * W], f32, tag=f"WZ{gi}", name=f"WZ{gi}")
            nc.scalar.activation(WZ[:, 4 * W:8 * W], LR[:],
                                 mybir.ActivationFunctionType.Exp)
            nc.gpsimd.tensor_tensor(WZ[:, 0:4 * W], WZ[:, 4 * W:8 * W],
                                    Sg[:, 4 * W:8 * W],
                                    op=mybir.AluOpType.mult)
            wv = WZ[:].rearrange("p (f t) -> p f t", f=8, t=W)
            for (b0, b1, C) in grp:
                nb = b1 - b0
                j0 = offs[b0] - o0g
                nc.vector.reduce_sum(
                    s8v[:, b0:b1, :],
                    wv[:, :, j0:j0 + nb * C]
                        .rearrange("p f (b c) -> p b f c", b=nb, c=C),
                    axis=mybir.AxisListType.X)
        # epilogue: h2[b] = sum_h num/den + c0   (den > 0 by host padding)
        dn = consts.tile([P, 4 * NB], f32)
        nc.vector.reciprocal_approx_fast(dn[:], s8v[:, :, 4:8])
        nc.vector.tensor_tensor(
            dn[:].rearrange("p (b h) -> p b h", b=NB, h=H),
            dn[:].rearrange("p (b h) -> p b h", b=NB, h=H),
            s8v[:, :, 0:4], op=mybir.AluOpType.mult)
        h2o = consts.tile([P, NB], f32)
        nc.vector.reduce_sum(
            h2o[:], dn[:].rearrange("p (b h) -> p b h", b=NB, h=H),
            axis=mybir.AxisListType.X)
        nc.vector.tensor_scalar(h2o[:], h2o[:], sc[0:P, 3:4], None,
                                op0=mybir.AluOpType.add)
        nc.sync.dma_start(t_h2.ap(), h2o[:])

    nc.compile()
    return nc


# --------------------------------------------------------------------------
# launch 2: per-slot h2 scalars -> output (layer-2 softmax + aggregate)
# --------------------------------------------------------------------------

def _build_l2(meta):
    from contextlib import ExitStack
    import concourse.tile as tile
    from concourse import bacc, mybir

    NB, T1 = meta["NB"], meta["T1"]
    CB, offs = meta["CB"], meta["offs"]
    n_cores = meta["n_cores"]
    f32, fp16 = mybir.dt.float32, mybir.dt.float16

    nc = bacc.Bacc("TRN2", target_bir_lowering=False, debug=False,
                   enable_asserts=False, num_devices=n_cores)
    t_gd = nc.dram_tensor("gd", [P, 2 * T1], fp16, kind="ExternalInput")
    t_sc = nc.dram_tensor("screp", [P, 4], f32, kind="ExternalInput")
    t_out = nc.dram_tensor("out", [P, NB], f32, kind="ExternalOutput")

    groups = _groups(_runs(CB), T1, 3, first_frac=0.5)

    with tile.TileContext(nc) as tc, ExitStack() as ctx:
        sb = ctx.enter_context(tc.tile_pool(name="sb", bufs=1))
        sc = sb.tile([P, 4], f32)
        nc.sync.dma_start(sc[:], t_sc.ap())
        alf = sb.tile([P, 1], f32)       # Prelu negative slope
        nc.vector.tensor_scalar(alf[:], sc[0:P, 0:1], 0.0, NEG_SLOPE,
                                op0=mybir.AluOpType.mult,
                                op1=mybir.AluOpType.add)
        s2 = sb.tile([P, 2 * NB], f32)   # block-major [num | den]
        s2v = s2[:].rearrange("p (b f) -> p b f", b=NB, f=2)

        sl = ctx.enter_context(tc.tile_pool(name="sl", bufs=1))

        tiles = []
        for gi, grp in enumerate(groups):
            b0g, b1g = grp[0][0], grp[-1][1]
            o0g, o1g = offs[b0g], offs[b1g]
            W = o1g - o0g
            GD = sl.tile([P, 2 * W], fp16, tag=f"GD{gi}", name=f"GD{gi}")
            nc.sync.dma_start(GD[:], t_gd.ap()[:, 2 * o0g:2 * o1g])
            g = GD[:, 0:W]
            d = GD[:, W:2 * W]
            u1 = sl.tile([P, W], f32, tag=f"u1{gi}", name=f"u1{gi}")
            nc.vector.tensor_tensor(
                u1[:], g, sc[0:P, 0:1].to_broadcast([P, W]),
                op=mybir.AluOpType.mult)
            nc.vector.scalar_tensor_tensor(
                u1[:], d, sc[0:P, 1:2], u1[:],
                op0=mybir.AluOpType.mult, op1=mybir.AluOpType.add)
            lr = sl.tile([P, W], fp16, tag=f"lr{gi}", name=f"lr{gi}")
            nc.scalar.activation(lr[:], u1[:],
                                 mybir.ActivationFunctionType.Prelu,
                                 alpha=alf[:])
            tiles.append((grp, o0g, W, g, lr))

        for gi, (grp, o0g, W, g, lr) in enumerate(tiles):
            WG = sl.tile([P, 2 * W], fp16, tag=f"WG{gi}", name=f"WG{gi}")
            nc.scalar.activation(WG[:, W:2 * W], lr[:],
                                 mybir.ActivationFunctionType.Exp)
            nc.vector.tensor_tensor(WG[:, 0:W], WG[:, W:2 * W], g,
                                    op=mybir.AluOpType.mult)
            wv = WG[:].rearrange("p (f t) -> p f t", f=2, t=W)
            for (b0, b1, C) in grp:
                nb = b1 - b0
                j0 = offs[b0] - o0g
                nc.vector.reduce_sum(
                    s2v[:, b0:b1, :],
                    wv[:, :, j0:j0 + nb * C]
                        .rearrange("p f (b c) -> p b f c", b=nb, c=C),
                    axis=mybir.AxisListType.X)
        dn = sb.tile([P, NB], f32)
        dn3 = dn[:].rearrange("p (b one) -> p b one", b=NB, one=1)
        nc.vector.reciprocal_approx_fast(dn3, s2v[:, :, 1:2])
        nc.vector.tensor_tensor(dn3, dn3, s2v[:, :, 0:1],
                                op=mybir.AluOpType.mult)
        nc.vector.tensor_scalar(dn[:], dn[:], sc[0:P, 2:3], None,
                                op0=mybir.AluOpType.add)
        nc.sync.dma_start(t_out.ap(), dn[:])

    nc.compile()
    return nc


# --------------------------------------------------------------------------
# entry point
# --------------------------------------------------------------------------

def _install_ntff_shim():
    """Optional: register the axon NTFF profiling hook (dev tracing only)."""
    import sys as _sys
    import types as _types
    if "antenv.axon_hooks" in _sys.modules:
        return
    try:
        import antenv
        mod = _types.ModuleType("antenv.axon_hooks")
        _state = {"hook": None}
        mod.set_axon_ntff_profile_hook = lambda h: _state.__setitem__("hook", h)
        mod.get_axon_ntff_profile_hook = lambda: _state["hook"]
        _sys.modules["antenv.axon_hooks"] = mod
        antenv.axon_hooks = mod
        from trn_agent_boot.trn_boot import _ntff_profile_via_ctypes
        mod.set_axon_ntff_profile_hook(
            _ntff_profile_via_ctypes("/opt/axon/libaxon_pjrt.so"))
    except Exception as e:  # pragma: no cover
        print("ntff shim unavailable:", e)


def kernel(**inputs):
    global LAST_EXEC_NS, LAST_RESULTS
    from concourse import bass_utils

    meta, shared, cores = _preprocess(**inputs)
    key = (meta["LP"], meta["T1"], tuple(meta["CB"]))
    if key not in _COMPILED:
        try:
            nc0 = _build_l0(meta, psum_dma=True)
        except Exception:
            nc0 = _build_l0(meta, psum_dma=False)
        _COMPILED[key] = (nc0, _build_l1(meta), _build_l2(meta))
    nc0, nc1, nc2 = _COMPILED[key]
    n_cores, LP, NB, T1 = meta["n_cores"], meta["LP"], meta["NB"], meta["T1"]
    N = meta["N"]
    offs = meta["offs"]
    groups1 = _groups(_runs(meta["CB"]), T1, 5, first_frac=0.5)
    groups2 = _groups(_runs(meta["CB"]), T1, 3, first_frac=0.5)

    trace = os.environ.get("GAT_TRACE", "0") == "1"
    if trace:
        _install_ntff_shim()

    wsc = np.asarray(shared["wsc"])
    screp = shared["screp"]

    # ---- launch 0: p = x @ wsc per node ----
    in0 = [{"xT": np.asarray(st["xT"]), "wsc": wsc} for st in cores]
    res0 = bass_utils.run_bass_kernel_spmd(
        nc0, in0, core_ids=list(range(n_cores)), trace=trace)

    p_node = np.zeros((N, PW), dtype=np.float32)
    for c in range(n_cores):
        pv = res0.results[c]["p"].reshape(P, NB, PW)
        pv = pv.transpose(1, 0, 2).reshape(LP, PW)
        pids = cores[c]["pids"]
        real = pids >= 0
        p_node[pids[real]] = pv[real]

    # ---- launch 1: layer-1 edge pass (plane-major group-packed fp16) ----
    in1 = []
    for c in range(n_cores):
        st = cores[c]
        Sv = p_node[st["sg"]]                      # [P, T1, 12]
        full = np.empty((P, T1, PW), dtype=np.float32)
        full[:, :, 0:HEADS] = np.where(st["valid"][:, :, None],
                                       Sv[:, :, 0:HEADS],
                                       np.float32(NEG_BIG))
        full[:, :, HEADS:8] = Sv[:, :, HEADS:8]
        full[:, :, 8:12] = p_node[st["dg"]][:, :, 8:12]
        # pad lanes: slot 0 of each block gets es=0, z=0 -> den=1, num=0
        # (keeps the device epilogue eps-free; pad h2 is discarded anyway)
        pids = st["pids"]
        lv = pids.reshape(NB, P).T >= 0            # [P, NB]
        pr, pb = np.nonzero(~lv)
        if pr.size:
            o_arr = np.asarray(offs[:-1])
            full[pr, o_arr[pb], 0:HEADS] = 0.0
            full[pr, o_arr[pb], HEADS:8] = 0.0
        parts = []
        for grp in groups1:
            o0, o1 = offs[grp[0][0]], offs[grp[-1][1]]
            parts.append(full[:, o0:o1, :].transpose(0, 2, 1).reshape(P, -1))
        S = np.concatenate(parts, axis=1).astype(np.float16)
        in1.append({"S": S, "screp": screp})
    res1 = bass_utils.run_bass_kernel_spmd(
        nc1, in1, core_ids=list(range(n_cores)), trace=trace)

    h2_node = np.zeros(N, dtype=np.float32)
    for c in range(n_cores):
        h2v = res1.results[c]["h2"]                # [P, NB]
        pids = cores[c]["pids"]
        real = pids >= 0
        h2_node[pids[real]] = h2v.T.reshape(-1)[real]

    # ---- launch 2: layer-2 edge pass (plane-major group-packed fp16) ----
    in2 = []
    a_s2 = float(screp[0, 0])
    a_d2 = float(screp[0, 1])
    # fp16-safe kill: |a_s2 * 60000| >> 1 makes exp(prelu(u)) underflow to 0
    if abs(a_s2) > 1e-8:
        kill_g, kill_d = -60000.0 * np.sign(a_s2), 0.0
    elif abs(a_d2) > 1e-8:
        kill_g, kill_d = 0.0, -60000.0 * np.sign(a_d2)
    else:
        kill_g, kill_d = 0.0, 0.0
    for c in range(n_cores):
        st = cores[c]
        g2 = np.where(st["valid"], h2_node[st["sg"]],
                      np.float32(kill_g)).astype(np.float32)
        dexp = np.where(st["valid"], h2_node[st["dg"]],
                        np.float32(kill_d)).astype(np.float32)
        parts = []
        for grp in groups2:
            o0, o1 = offs[grp[0][0]], offs[grp[-1][1]]
            parts.append(np.concatenate(
                [g2[:, o0:o1], dexp[:, o0:o1]], axis=1))
        gd = np.ascontiguousarray(
            np.concatenate(parts, axis=1)).astype(np.float16)
        in2.append({"gd": gd, "screp": screp})
    res2 = bass_utils.run_bass_kernel_spmd(
        nc2, in2, core_ids=list(range(n_cores)), trace=trace)

    ts = [r.exec_time_ns or 0 for r in (res0, res1, res2)]
    LAST_EXEC_NS = sum(ts) if any(ts) else None
    LAST_RESULTS = (res0, res1, res2)

    out = np.zeros((N, 1), dtype=np.float32)
    for c in range(n_cores):
        vals = res2.results[c]["out"]              # [P, NB]
        pids = cores[c]["pids"]
        real = pids >= 0
        out[pids[real], 0] = vals.T.reshape(-1)[real]
    return out


# revision 27
# speedup vs baseline: 1.0906x; 1.0906x over previous
"""Bass/Trainium2 kernel for 2-layer GAT (nn_GATa_45260365365735).

Strategy (8 NeuronCores, SPMD, three launches; host does only
indexing/duplication between launches — all arithmetic on device):

  - Nodes assigned to cores round-robin by global in-degree rank (balanced
    edges + identical block shapes).  Each core owns all edges targeting its
    nodes; segment softmax + aggregation are core-local.  Owned nodes form
    128-lane blocks; lane (p, b) holds its node's in-edges in chunk columns.
  - Algebraic collapse: layer 2 only consumes h2 = h1 @ W2; by linearity a
    node only needs p = x @ wsc with 12 outputs: e_src(4) | z(4) | e_dst(4),
    where z folds W2 into W1.
  - Launch 0: p per NODE ([N,128] @ [128,12] bf16) — removes the per-edge
    duplication of 128-dim features (23 MB/core -> 3.2 MB/core HBM).
  - Host gathers per-slot fp16 streams in PLANE-MAJOR, GROUP-PACKED layout:
    every elementwise op and segmented reduction runs on contiguous data.
  - Launch 1 (edge pass) engine split tuned to measured rates: gpsimd does
    u = e_src + e_dst; the scalar engine does ALL Prelus then ALL Exps (two
    activation-table loads total; fp16 outputs); vector does w*z (fp16, 2x
    rate) + per-run 4D segmented reductions (runs of equal chunk count C are
    contiguous since CB is non-increasing) + the epilogue with
    reciprocal_approx_fast.  Pad slots: host writes e_src = -60000 -> w = 0.
  - Launch 2 repeats the pattern on h2 streams (1 "head"); output written
    contiguously as [P, NB] (host inverse-permutes).
"""

import os
import numpy as np
import ml_dtypes

P = 128
N_CORES = 8
HEADS = 4
HID = 32
IN_DIM = 128
NEG_SLOPE = 0.2
EPS = 1e-16
PW = 12            # per-node payload: e_src(4) | z(4) | e_dst(4)
NEG_BIG = -60000.0   # pad-slot e_src kill value (fp16-safe)

_COMPILED = {}
LAST_EXEC_NS = None
LAST_RESULTS = None


# --------------------------------------------------------------------------
# host preprocessing (indexing only)
# --------------------------------------------------------------------------

def _preprocess(x, edge_index, W1, att_src1, att_dst1, b1, W2, att_src2,
                att_dst2, b2, n_cores=None):
    if n_cores is None:
        n_cores = N_CORES
    N = x.shape[0]
    ei = np.asarray(edge_index).astype(np.int64)
    src = np.concatenate([ei[0], np.arange(N, dtype=np.int64)])
    dst = np.concatenate([ei[1], np.arange(N, dtype=np.int64)])

    deg = np.bincount(dst, minlength=N).astype(np.int64)
    order = np.argsort(dst, kind="stable")
    src_sorted = src[order]
    estart = np.concatenate([[0], np.cumsum(deg)]).astype(np.int64)

    grank = np.argsort(-deg, kind="stable")
    perms = [grank[c::n_cores] for c in range(n_cores)]
    LP = int(np.ceil(max(len(p) for p in perms) / P) * P)
    NB = LP // P
    for c in range(n_cores):
        pad = np.full(LP - len(perms[c]), -1, dtype=np.int64)
        perms[c] = np.concatenate([perms[c], pad])

    blockmax = np.zeros((n_cores, NB), dtype=np.int64)
    for c in range(n_cores):
        pids = perms[c]
        d = np.where(pids >= 0, deg[np.maximum(pids, 0)], 0)
        blockmax[c] = d.reshape(NB, P).max(axis=1)
    CB = np.maximum(blockmax.max(axis=0), 1).astype(np.int64)
    T1 = int(CB.sum())
    offs = np.concatenate([[0], np.cumsum(CB)]).astype(np.int64)

    xf = np.asarray(x, dtype=np.float32)
    cores = []
    for c in range(n_cores):
        pids = perms[c]
        safe_pids = np.maximum(pids, 0)
        sg = np.zeros((P, T1), dtype=np.int64)   # src node per slot
        dg = np.zeros((P, T1), dtype=np.int64)   # dst (lane) node per slot
        valid = np.zeros((P, T1), dtype=bool)
        for b in range(NB):
            C = int(CB[b])
            rows = pids[b * P:(b + 1) * P]
            safe = np.maximum(rows, 0)
            d = np.where(rows >= 0, deg[safe], 0)
            st = estart[safe]
            cols = np.arange(C, dtype=np.int64)[None, :]
            vb = cols < d[:, None]
            eix = np.clip(st[:, None] + cols, 0, src.shape[0] - 1)
            o0 = int(offs[b])
            sg[:, o0:o0 + C] = np.where(vb, src_sorted[eix], 0)
            dg[:, o0:o0 + C] = np.where(vb, safe[:, None], 0)
            valid[:, o0:o0 + C] = vb
        xTc = np.ascontiguousarray(
            xf[safe_pids].T).astype(ml_dtypes.bfloat16)   # [IN_DIM, LP]
        cores.append(dict(xT=xTc, sg=sg, dg=dg, valid=valid, pids=pids))

    W1 = np.asarray(W1, dtype=np.float32)
    a_s1 = np.asarray(att_src1, dtype=np.float32)
    a_d1 = np.asarray(att_dst1, dtype=np.float32)
    W2v = np.asarray(W2, dtype=np.float32).reshape(-1)
    W1a = np.einsum("khc,hc->kh", W1.reshape(IN_DIM, HEADS, HID), a_s1)
    W1b = np.einsum("khc,hc->kh", W1.reshape(IN_DIM, HEADS, HID), a_d1)
    W12h = np.einsum("khf,hf->kh", W1.reshape(IN_DIM, HEADS, HID),
                     W2v.reshape(HEADS, HID))
    wsc = np.concatenate([W1a, W12h, W1b], axis=1).astype(ml_dtypes.bfloat16)
    b1v = np.asarray(b1, dtype=np.float32).reshape(-1)
    c0 = float(b1v @ W2v)
    screp = np.zeros((P, 4), dtype=np.float32)
    screp[:, 0] = float(np.asarray(att_src2).reshape(-1)[0])
    screp[:, 1] = float(np.asarray(att_dst2).reshape(-1)[0])
    screp[:, 2] = float(np.asarray(b2).reshape(-1)[0])
    screp[:, 3] = c0

    meta = dict(N=N, LP=LP, NB=NB, T1=T1, CB=CB.tolist(),
                offs=offs.tolist(), n_cores=n_cores)
    shared = dict(wsc=wsc, screp=screp)
    return meta, shared, cores


def _runs(CB):
    """Contiguous runs of equal chunk count (CB is non-increasing)."""
    runs = []
    b = 0
    NB = len(CB)
    while b < NB:
        e = b
        while e < NB and CB[e] == CB[b]:
            e += 1
        runs.append((b, e, CB[b]))
        b = e
    return runs


def _groups(runs, T1, n, first_frac=1.0):
    """Split runs into ~n groups (run-aligned); first group scaled by
    first_frac so the compute pipeline starts on a smaller DMA."""
    weights = [first_frac] + [1.0] * (n - 1)
    wsum = sum(weights)
    groups, cur, tot = [], [], 0
    gi = 0
    tgt = T1 * weights[0] / wsum
    for r in runs:
        cur.append(r)
        tot += (r[1] - r[0]) * r[2]
        if tot >= tgt and gi < n - 1:
            groups.append(cur)
            cur, tot = [], 0
            gi += 1
            tgt = T1 * weights[gi] / wsum
    if cur:
        groups.append(cur)
    return groups


def _l0_groups(NB):
    sizes = []
    left = NB
    for s in [8, 18, 24]:
        if left <= 0:
            break
        sizes.append(min(s, left))
        left -= sizes[-1]
    while left > 0:
        s = min(24, left)
        sizes.append(s)
        left -= s
    return sizes


# --------------------------------------------------------------------------
# launch 0: per-node p = x @ wsc   (x sharded by owned nodes)
# --------------------------------------------------------------------------

def _build_l0(meta, psum_dma=True):
    from contextlib import ExitStack
    import concourse.tile as tile
    from concourse import bacc, mybir

    LP, NB = meta["LP"], meta["NB"]
    n_cores = meta["n_cores"]
    f32, bf16 = mybir.dt.float32, mybir.dt.bfloat16

    nc = bacc.Bacc("TRN2", target_bir_lowering=False, debug=False,
                   enable_asserts=False, num_devices=n_cores)
    t_xT = nc.dram_tensor("xT", [IN_DIM, LP], bf16, kind="ExternalInput")
    t_wsc = nc.dram_tensor("wsc", [IN_DIM, PW], bf16, kind="ExternalInput")
    t_p = nc.dram_tensor("p", [P, NB * PW], f32, kind="ExternalOutput")

    sizes = _l0_groups(NB)

    with tile.TileContext(nc) as tc, ExitStack() as ctx:
        consts = ctx.enter_context(tc.tile_pool(name="consts", bufs=1))
        wsct = consts.tile([IN_DIM, PW], bf16)
        nc.sync.dma_start(wsct[:], t_wsc.ap())

        xp = ctx.enter_context(tc.tile_pool(name="xp", bufs=1))
        pp = ctx.enter_context(tc.tile_pool(name="pp", bufs=4, space="PSUM"))
        op = ctx.enter_context(tc.tile_pool(name="op", bufs=1))

        # issue every input DMA first so the sync queue never stalls on
        # an output DMA that waits for compute
        xts = []
        b0 = 0
        for gi, nbg in enumerate(sizes):
            xt = xp.tile([P, nbg * P], bf16, tag=f"xt{gi}", name=f"xt{gi}")
            nc.sync.dma_start(
                xt[:], t_xT.ap()[:, b0 * P:(b0 + nbg) * P])
            xts.append((xt, b0, nbg))
            b0 += nbg

        for gi, (xt, b0, nbg) in enumerate(xts):
            ps = pp.tile([P, nbg * PW], f32, tag="ps",
                         padded_shape=[P, 24 * PW], name=f"ps{gi}")
            for j in range(nbg):
                nc.tensor.matmul(ps[:, j * PW:(j + 1) * PW],
                                 lhsT=xt[:, j * P:(j + 1) * P],
                                 rhs=wsct[:], start=True, stop=True)
            if psum_dma:
                nc.sync.dma_start(
                    t_p.ap()[:, b0 * PW:(b0 + nbg) * PW], ps[:, 0:nbg * PW])
            else:
                po = op.tile([P, nbg * PW], f32, tag=f"po{gi}",
                             name=f"po{gi}")
                nc.vector.tensor_copy(po[:], ps[:, 0:nbg * PW])
                nc.sync.dma_start(
                    t_p.ap()[:, b0 * PW:(b0 + nbg) * PW], po[:])

    nc.compile()
    return nc


# --------------------------------------------------------------------------
# launch 1: per-slot p streams -> per-node h2 (layer-1 softmax + aggregate)
# --------------------------------------------------------------------------

def _build_l1(meta):
    from contextlib import ExitStack
    import concourse.tile as tile
    from concourse import bacc, mybir

    NB, T1 = meta["NB"], meta["T1"]
    CB, offs = meta["CB"], meta["offs"]
    n_cores = meta["n_cores"]
    f32, fp16 = mybir.dt.float32, mybir.dt.float16
    H = HEADS

    nc = bacc.Bacc("TRN2", target_bir_lowering=False, debug=False,
                   enable_asserts=False, num_devices=n_cores)
    t_S = nc.dram_tensor("S", [P, T1 * PW], fp16, kind="ExternalInput")
    t_sc = nc.dram_tensor("screp", [P, 4], f32, kind="ExternalInput")
    t_h2 = nc.dram_tensor("h2", [P, NB], f32, kind="ExternalOutput")

    groups = _groups(_runs(CB), T1, 5, first_frac=0.5)

    with tile.TileContext(nc) as tc, ExitStack() as ctx:
        consts = ctx.enter_context(tc.tile_pool(name="consts", bufs=1))
        sc = consts.tile([P, 4], f32)
        nc.sync.dma_start(sc[:], t_sc.ap())
        alf = consts.tile([P, 1], f32)       # Prelu negative slope
        nc.vector.tensor_scalar(alf[:], sc[0:P, 0:1], 0.0, NEG_SLOPE,
                                op0=mybir.AluOpType.mult,
                                op1=mybir.AluOpType.add)
        s8 = consts.tile([P, 8 * NB], f32)   # block-major [num(4) | den(4)]
        s8v = s8[:].rearrange("p (b f) -> p b f", b=NB, f=8)

        sp = ctx.enter_context(tc.tile_pool(name="sp", bufs=1))

        tiles = []
        # phase A: DMA + u-add (gpsimd) + Prelu (scalar, one table)
        for gi, grp in enumerate(groups):
            b0g, b1g = grp[0][0], grp[-1][1]
            o0g, o1g = offs[b0g], offs[b1g]
            W = o1g - o0g
            Sg = sp.tile([P, W * PW], fp16, tag=f"S{gi}", name=f"S{gi}")
            nc.sync.dma_start(Sg[:], t_S.ap()[:, o0g * PW:o1g * PW])
            U = sp.tile([P, 4 * W], fp16, tag=f"U{gi}", name=f"U{gi}")
            nc.vector.tensor_tensor(U[:], Sg[:, 0:4 * W], Sg[:, 8 * W:12 * W],
                                    op=mybir.AluOpType.add)
            LR = sp.tile([P, 4 * W], fp16, tag=f"LR{gi}", name=f"LR{gi}")
            nc.scalar.activation(LR[:], U[:],
                                 mybir.ActivationFunctionType.Prelu,
                                 alpha=alf[:])
            tiles.append((grp, o0g, W, Sg, LR))

        # phase B: Exp (scalar, one table) + w*z (vector) + reductions
        for gi, (grp, o0g, W, Sg, LR) in enumerate(tiles):
            WZ = sp.tile([P, 8 * W], fp16, tag=f"WZ{gi}", name=f"WZ{gi}")
            nc.scalar.activation(WZ[:, 4 * W:8 * W], LR[:],
                                 mybir.ActivationFunctionType.Exp)
            nc.vector.tensor_tensor(WZ[:, 0:4 * W], WZ[:, 4 * W:8 * W],
                                    Sg[:, 4 * W:8 * W],
                                    op=mybir.AluOpType.mult)
            wv = WZ[:].rearrange("p (f t) -> p f t", f=8, t=W)
            for (b0, b1, C) in grp:
                nb = b1 - b0
                j0 = offs[b0] - o0g
                nc.vector.reduce_sum(
                    s8v[:, b0:b1, :],
                    wv[:, :, j0:j0 + nb * C]
                        .rearrange("p f (b c) -> p b f c", b=nb, c=C),
                    axis=mybir.AxisListType.X)
        # epilogue: h2[b] = sum_h num/den + c0   (den > 0 by host padding)
        dn = consts.tile([P, 4 * NB], f32)
        nc.vector.reciprocal_approx_fast(dn[:], s8v[:, :, 4:8])
        nc.vector.tensor_tensor(
            dn[:].rearrange("p (b h) -> p b h", b=NB, h=H),
            dn[:].rearrange("p (b h) -> p b h", b=NB, h=H),
            s8v[:, :, 0:4], op=mybir.AluOpType.mult)
        h2o = consts.tile([P, NB], f32)
        nc.vector.reduce_sum(
            h2o[:], dn[:].rearrange("p (b h) -> p b h", b=NB, h=H),
            axis=mybir.AxisListType.X)
        nc.vector.tensor_scalar(h2o[:], h2o[:], sc[0:P, 3:4], None,
                                op0=mybir.AluOpType.add)
        nc.sync.dma_start(t_h2.ap(), h2o[:])

    nc.compile()
    return nc


# --------------------------------------------------------------------------
# launch 2: per-slot h2 scalars -> output (layer-2 softmax + aggregate)
# --------------------------------------------------------------------------

def _build_l2(meta):
    from contextlib import ExitStack
    import concourse.tile as tile
    from concourse import bacc, mybir

    NB, T1 = meta["NB"], meta["T1"]
    CB, offs = meta["CB"], meta["offs"]
    n_cores = meta["n_cores"]
    f32, fp16 = mybir.dt.float32, mybir.dt.float16

    nc = bacc.Bacc("TRN2", target_bir_lowering=False, debug=False,
                   enable_asserts=False, num_devices=n_cores)
    t_gd = nc.dram_tensor("gd", [P, 2 * T1], fp16, kind="ExternalInput")
    t_sc = nc.dram_tensor("screp", [P, 4], f32, kind="ExternalInput")
    t_out = nc.dram_tensor("out", [P, NB], f32, kind="ExternalOutput")

    groups = _groups(_runs(CB), T1, 3, first_frac=0.5)

    with tile.TileContext(nc) as tc, ExitStack() as ctx:
        sb = ctx.enter_context(tc.tile_pool(name="sb", bufs=1))
        sc = sb.tile([P, 4], f32)
        nc.sync.dma_start(sc[:], t_sc.ap())
        alf = sb.tile([P, 1], f32)       # Prelu negative slope
        nc.vector.tensor_scalar(alf[:], sc[0:P, 0:1], 0.0, NEG_SLOPE,
                                op0=mybir.AluOpType.mult,
                                op1=mybir.AluOpType.add)
        s2 = sb.tile([P, 2 * NB], f32)   # block-major [num | den]
        s2v = s2[:].rearrange("p (b f) -> p b f", b=NB, f=2)

        sl = ctx.enter_context(tc.tile_pool(name="sl", bufs=1))

        tiles = []
        for gi, grp in enumerate(groups):
            b0g, b1g = grp[0][0], grp[-1][1]
            o0g, o1g = offs[b0g], offs[b1g]
            W = o1g - o0g
            GD = sl.tile([P, 2 * W], fp16, tag=f"GD{gi}", name=f"GD{gi}")
            nc.sync.dma_start(GD[:], t_gd.ap()[:, 2 * o0g:2 * o1g])
            g = GD[:, 0:W]
            d = GD[:, W:2 * W]
            u1 = sl.tile([P, W], f32, tag=f"u1{gi}", name=f"u1{gi}")
            nc.vector.tensor_tensor(
                u1[:], g, sc[0:P, 0:1].to_broadcast([P, W]),
                op=mybir.AluOpType.mult)
            nc.vector.scalar_tensor_tensor(
                u1[:], d, sc[0:P, 1:2], u1[:],
                op0=mybir.AluOpType.mult, op1=mybir.AluOpType.add)
            lr = sl.tile([P, W], fp16, tag=f"lr{gi}", name=f"lr{gi}")
            nc.scalar.activation(lr[:], u1[:],
                                 mybir.ActivationFunctionType.Prelu,
                                 alpha=alf[:])
            tiles.append((grp, o0g, W, g, lr))

        for gi, (grp, o0g, W, g, lr) in enumerate(tiles):
            WG = sl.tile([P, 2 * W], fp16, tag=f"WG{gi}", name=f"WG{gi}")
            nc.scalar.activation(WG[:, W:2 * W], lr[:],
                                 mybir.ActivationFunctionType.Exp)
            nc.vector.tensor_tensor(WG[:, 0:W], WG[:, W:2 * W], g,
                                    op=mybir.AluOpType.mult)
            wv = WG[:].rearrange("p (f t) -> p f t", f=2, t=W)
            for (b0, b1, C) in grp:
                nb = b1 - b0
                j0 = offs[b0] - o0g
                nc.vector.reduce_sum(
                    s2v[:, b0:b1, :],
                    wv[:, :, j0:j0 + nb * C]
                        .rearrange("p f (b c) -> p b f c", b=nb, c=C),
                    axis=mybir.AxisListType.X)
        dn = sb.tile([P, NB], f32)
        dn3 = dn[:].rearrange("p (b one) -> p b one", b=NB, one=1)
        nc.vector.reciprocal_approx_fast(dn3, s2v[:, :, 1:2])
        nc.vector.tensor_tensor(dn3, dn3, s2v[:, :, 0:1],
                                op=mybir.AluOpType.mult)
        nc.vector.tensor_scalar(dn[:], dn[:], sc[0:P, 2:3], None,
                                op0=mybir.AluOpType.add)
        nc.sync.dma_start(t_out.ap(), dn[:])

    nc.compile()
    return nc


# --------------------------------------------------------------------------
# entry point
# --------------------------------------------------------------------------

def _install_ntff_shim():
    """Optional: register the axon NTFF profiling hook (dev tracing only)."""
    import sys as _sys
    import types as _types
    if "antenv.axon_hooks" in _sys.modules:
        return
    try:
        import antenv
        mod = _types.ModuleType("antenv.axon_hooks")
        _state = {"hook": None}
        mod.set_axon_ntff_profile_hook = lambda h: _state.__setitem__("hook", h)
        mod.get_axon_ntff_profile_hook = lambda: _state["hook"]
        _sys.modules["antenv.axon_hooks"] = mod
        antenv.axon_hooks = mod
        from trn_agent_boot.trn_boot import _ntff_profile_via_ctypes
        mod.set_axon_ntff_profile_hook(
            _ntff_profile_via_ctypes("/opt/axon/libaxon_pjrt.so"))
    except Exception as e:  # pragma: no cover
        print("ntff shim unavailable:", e)


def kernel(**inputs):
    global LAST_EXEC_NS, LAST_RESULTS
    from concourse import bass_utils

    meta, shared, cores = _preprocess(**inputs)
    key = (meta["LP"], meta["T1"], tuple(meta["CB"]))
    if key not in _COMPILED:
        try:
            nc0 = _build_l0(meta, psum_dma=True)
        except Exception:
            nc0 = _build_l0(meta, psum_dma=False)
        _COMPILED[key] = (nc0, _build_l1(meta), _build_l2(meta))
    nc0, nc1, nc2 = _COMPILED[key]
    n_cores, LP, NB, T1 = meta["n_cores"], meta["LP"], meta["NB"], meta["T1"]
    N = meta["N"]
    offs = meta["offs"]
    groups1 = _groups(_runs(meta["CB"]), T1, 5, first_frac=0.5)
    groups2 = _groups(_runs(meta["CB"]), T1, 3, first_frac=0.5)

    trace = os.environ.get("GAT_TRACE", "0") == "1"
    if trace:
        _install_ntff_shim()

    wsc = np.asarray(shared["wsc"])
    screp = shared["screp"]

    # ---- launch 0: p = x @ wsc per node ----
    in0 = [{"xT": np.asarray(st["xT"]), "wsc": wsc} for st in cores]
    res0 = bass_utils.run_bass_kernel_spmd(
        nc0, in0, core_ids=list(range(n_cores)), trace=trace)

    p_node = np.zeros((N, PW), dtype=np.float32)
    for c in range(n_cores):
        pv = res0.results[c]["p"].reshape(P, NB, PW)
        pv = pv.transpose(1, 0, 2).reshape(LP, PW)
        pids = cores[c]["pids"]
        real = pids >= 0
        p_node[pids[real]] = pv[real]

    # ---- launch 1: layer-1 edge pass (plane-major group-packed fp16) ----
    in1 = []
    for c in range(n_cores):
        st = cores[c]
        Sv = p_node[st["sg"]]                      # [P, T1, 12]
        full = np.empty((P, T1, PW), dtype=np.float32)
        full[:, :, 0:HEADS] = np.where(st["valid"][:, :, None],
                                       Sv[:, :, 0:HEADS],
                                       np.float32(NEG_BIG))
        full[:, :, HEADS:8] = Sv[:, :, HEADS:8]
        full[:, :, 8:12] = p_node[st["dg"]][:, :, 8:12]
        # pad lanes: slot 0 of each block gets es=0, z=0 -> den=1, num=0
        # (keeps the device epilogue eps-free; pad h2 is discarded anyway)
        pids = st["pids"]
        lv = pids.reshape(NB, P).T >= 0            # [P, NB]
        pr, pb = np.nonzero(~lv)
        if pr.size:
            o_arr = np.asarray(offs[:-1])
            full[pr, o_arr[pb], 0:HEADS] = 0.0
            full[pr, o_arr[pb], HEADS:8] = 0.0
        parts = []
        for grp in groups1:
            o0, o1 = offs[grp[0][0]], offs[grp[-1][1]]
            parts.append(full[:, o0:o1, :].transpose(0, 2, 1).reshape(P, -1))
        S = np.concatenate(parts, axis=1).astype(np.float16)
        in1.append({"S": S, "screp": screp})
    res1 = bass_utils.run_bass_kernel_spmd(
        nc1, in1, core_ids=list(range(n_cores)), trace=trace)

    h2_node = np.zeros(N, dtype=np.float32)
    for c in range(n_cores):
        h2v = res1.results[c]["h2"]                # [P, NB]
        pids = cores[c]["pids"]
        real = pids >= 0
        h2_node[pids[real]] = h2v.T.reshape(-1)[real]

    # ---- launch 2: layer-2 edge pass (plane-major group-packed fp16) ----
    in2 = []
    a_s2 = float(screp[0, 0])
    a_d2 = float(screp[0, 1])
    # fp16-safe kill: |a_s2 * 60000| >> 1 makes exp(prelu(u)) underflow to 0
    if abs(a_s2) > 1e-8:
        kill_g, kill_d = -60000.0 * np.sign(a_s2), 0.0
    elif abs(a_d2) > 1e-8:
        kill_g, kill_d = 0.0, -60000.0 * np.sign(a_d2)
    else:
        kill_g, kill_d = 0.0, 0.0
    for c in range(n_cores):
        st = cores[c]
        g2 = np.where(st["valid"], h2_node[st["sg"]],
                      np.float32(kill_g)).astype(np.float32)
        dexp = np.where(st["valid"], h2_node[st["dg"]],
                        np.float32(kill_d)).astype(np.float32)
        parts = []
        for grp in groups2:
            o0, o1 = offs[grp[0][0]], offs[grp[-1][1]]
            parts.append(np.concatenate(
                [g2[:, o0:o1], dexp[:, o0:o1]], axis=1))
        gd = np.ascontiguousarray(
            np.concatenate(parts, axis=1)).astype(np.float16)
        in2.append({"gd": gd, "screp": screp})
    res2 = bass_utils.run_bass_kernel_spmd(
        nc2, in2, core_ids=list(range(n_cores)), trace=trace)

    ts = [r.exec_time_ns or 0 for r in (res0, res1, res2)]
    LAST_EXEC_NS = sum(ts) if any(ts) else None
    LAST_RESULTS = (res0, res1, res2)

    out = np.zeros((N, 1), dtype=np.float32)
    for c in range(n_cores):
        vals = res2.results[c]["out"]              # [P, NB]
        pids = cores[c]["pids"]
        real = pids >= 0
        out[pids[real], 0] = vals.T.reshape(-1)[real]
    return out
